# revision 3
# baseline (speedup 1.0000x reference)
"""MultiModalPyramidAttentionFusion — Trainium2 Bass/Tile kernel.

Full inputs in, full output out. 8-way SPMD over (batch b) x (query-pixel
quarter q); each core computes the fused output for its 576 query pixels.
K/V projections (full 2304-pixel image) are replicated across the 4 cores
of a batch element — no collectives.

v2 attention core (vs v1): query tiles are 192 wide so QK logit chunks
pack two-to-a-PSUM-bank; the attention unit is a (head-pair, key-chunk-
pair) group. Per group: 4 QK matmuls (N=192) alternate head parity so the
two heads' K=64 matmuls run row-tile-CONCURRENT in the PE array (base
partitions 0/64), one fused EXP over all 4 chunks (FD=768, 3D AP — ACT
cost 260+FD/1.2 so fusing 4 chunks amortizes the fixed cost), and one
fp8-e4m3 DoubleRow AV matmul per head that contracts both 128-key chunks
in a single N=192 stream (2 fp8 MACs/cell/cycle). EXP writes fp8 directly;
V is stored fp8 with a ones column so softmax denominators fall out of the
same matmul. Both heads' AV accumulators pack into ONE PSUM bank
([128,2,192@256]), so the whole steady state fits 8 banks:
st 2x2 + ot 2x1 + filler 2x1.

Precision: projection path in bf16; P and V in fp8e4 (P in [0.67,1.5] so
e4m3 rel err ~3%, washes out over 2304-key softmax averaging; V errors
average the same way). K-projection bias dropped (cancels in softmax).
LN stats fp32; rstd via DVE Newton rsqrt.

Independent matmul work (modal-2 K/V projections, output projection / LN
stats / fusion) drains as PE filler between groups, same as v1.
"""

import os
from contextlib import ExitStack

import numpy as np

import concourse.bass as bass
import concourse.mybir as mybir
import concourse.tile as tile
from concourse import bacc
from concourse._compat import with_exitstack

F32 = mybir.dt.float32
F32R = mybir.dt.float32r
BF16 = mybir.dt.bfloat16
FP8 = mybir.dt.float8e4
AF = mybir.ActivationFunctionType
ALU = mybir.AluOpType
PM = mybir.MatmulPerfMode

B, C, H, W = 2, 256, 48, 48
L = H * W            # 2304
HID, NH, D = 512, 8, 64
EPS = 1e-5
SCALE = D ** -0.5    # 1/8

NCORES = 8
# Schraudolph exp for DVE offload: one tensor_scalar computes
# u = x*(SCALE*log2e*8) + (2^23 + 56 - 0.347) in fp32; the +2^23 forces
# round-to-int in the mantissa and the LOW 8 bits of the fp32 word are
# exactly the fp8e4m3 bits of 2^z (max rel err ~9%, washes out in the
# 2304-key softmax average).
WSC = 32.0           # host scale on wq/wk/wv so fp8e4 stays in normal range
LSC = 1.0 / (WSC * WSC)   # logit descale, folded into the EXP scale
EXPA8 = float(np.log2(np.e) * LSC)    # SCALE * log2e * 8 * LSC
EXPC8 = 8388663.653
# chunk-pair indices whose exp runs on the DVE instead of ACT (per tile>=1)
DVE_CPS = frozenset(
    int(x) for x in os.environ.get("MMPAF_DVE_CPS", "3").split(",") if x)
LQ = L // 4          # 576 query pixels per core
NT = 3               # query tiles per core per modal
TQ = LQ // NT        # 192-wide query tiles
NK = L // 128        # 18 key chunks
NP = NK // 2         # 9 key-chunk pairs
CC = C // 128        # 2 channel chunks
HC = HID // 128      # 4 hidden chunks
KT = 384             # free-tile for k projection (L = 6*384)
TQP = 288            # projection round width for q (LQ = 2*288)

# packed per-partition parameter layout: name -> (col offset, chunks)
_PARAM_SLOTS = {}
_off = 0
for _nm, _ch in [("bq1", 4), ("bq2", 4),
                 ("bo1p", 2), ("bo2p", 2), ("ln1g", 2), ("ln1b", 2),
                 ("ln2g", 2), ("ln2b", 2), ("bnw", 2), ("bnb", 2)]:
    _PARAM_SLOTS[_nm] = (_off, _ch)
    _off += _ch
NPARAM_COLS = _off  # 24

LAST_EXEC_NS = None
LAST_RESULTS = None


@with_exitstack
def core_kernel(ctx: ExitStack, tc: tile.TileContext, outs, ins):
    nc = tc.nc
    y_out = outs["y"]  # [256, 576]

    # ---------------- pools ----------------
    consts = ctx.enter_context(tc.tile_pool(name="consts", bufs=1))
    big = ctx.enter_context(tc.tile_pool(name="big", bufs=1))
    ptp = ctx.enter_context(tc.tile_pool(name="ptp", bufs=4))
    epi = ctx.enter_context(tc.tile_pool(name="epi", bufs=2))
    tmp = ctx.enter_context(tc.tile_pool(name="tmp", bufs=2))

    st_pool = ctx.enter_context(tc.tile_pool(name="st", bufs=2, space="PSUM"))
    ot_pool = ctx.enter_context(tc.tile_pool(name="ot", bufs=2, space="PSUM"))

    # ---------------- params (single packed DMA) ----------------
    params = consts.tile([128, NPARAM_COLS], F32)
    nc.gpsimd.dma_start(params[:], ins["params"][:])

    def prm(name):
        off, ch = _PARAM_SLOTS[name]
        return params[:, off:off + ch]

    bq_s = {1: prm("bq1"), 2: prm("bq2")}
    bo_s = {1: prm("bo1p"), 2: prm("bo2p")}
    lng_s = {1: prm("ln1g"), 2: prm("ln2g")}
    lnb_s = {1: prm("ln1b"), 2: prm("ln2b")}
    bnw_s, bnb_s = prm("bnw"), prm("bnb")

    ones_f32 = consts.tile([128, 128], F32)
    nc.vector.memset(ones_f32[:], 1.0 / C)
    ones_inv = consts.tile([128, 128], F32R)
    nc.vector.tensor_copy(ones_inv[:], ones_f32[:])
    # 1/WSC here un-scales the x32 V values through the denominator broadcast
    ones_bc = consts.tile([128, 64], BF16)
    nc.vector.memset(ones_bc[:], 1.0 / WSC)

    # ---------------- big SBUF tensors ----------------
    qT = {m: big.tile([128, HC, LQ], BF16, tag=f"qT{m}", name=f"qT{m}")
          for m in (1, 2)}
    kT = {m: big.tile([128, HC, L], BF16, tag=f"kT{m}", name=f"kT{m}")
          for m in (1, 2)}
    # V (+ones col) in fp8, keyed [chunk-pair, chunk-parity, head, 65@80]
    va = {m: big.tile([128, NP, 2, NH, 65], FP8,
                      padded_shape=[128, NP, 2, NH, 80],
                      tag=f"va{m}", name=f"va{m}")
          for m in (1, 2)}
    FP8E5 = mybir.dt.float8e5
    ost = {m: big.tile([128, HC, LQ], FP8E5, tag=f"ost{m}", name=f"ost{m}")
           for m in (1, 2)}
    msb = {m: big.tile([128, CC, LQ], BF16, tag=f"m{m}", name=f"msb{m}")
           for m in (1, 2)}
    xq = {}
    xq8 = {}
    for m in (1, 2):
        xq[m] = big.tile([128, CC, LQ], BF16, tag=f"xq{m}", name=f"xq{m}")
        xq8[m] = big.tile([128, CC, LQ], FP8, tag=f"xq8{m}", name=f"xq8{m}")

    woT = {}
    for m in (1, 2):
        woT[m] = big.tile([128, HC, C], FP8E5, tag=f"woT{m}", name=f"woT{m}")
        nc.gpsimd.dma_start(
            woT[m][:], ins[f"wo{m}T"].rearrange("(a p) c -> p a c", p=128)
        )
    wfT = big.tile([128, HC, C], BF16, tag="wfT")
    nc.gpsimd.dma_start(wfT[:], ins["wfT"].rearrange("(a p) c -> p a c", p=128))

    # ---------------- filler machinery ----------------
    fillers = []      # closures of independent PE work, drained in attention

    def fill(n):
        for _ in range(n):
            if not fillers:
                return
            fillers.pop(0)()

    # ---------------- projections ----------------
    def open_w(ctx2, m, dma=True):
        wp = ctx2.enter_context(tc.tile_pool(name=f"wp{m}", bufs=1))
        ws = []
        for wn in ("wq", "wk", "wv"):
            w = wp.tile([128, CC, HID], FP8, tag=wn, name=f"{wn}{m}")
            if dma:
                dma_w(w, wn, m)
            ws.append(w)
        return ws

    def dma_w(w, wn, m):
        nc.sync.dma_start(
            w[:], ins[f"{wn}{m}T"].rearrange("(a p) h -> p a h", p=128)
        )

    def open_x(ctx2, m, mid=None, queue=None, nch=2):
        xf = ctx2.enter_context(tc.tile_pool(name=f"xf{m}", bufs=1))
        x_full = xf.tile([128, CC, L], FP8, tag="xfull", name=f"xfull{m}")
        src = ins[f"x{m}f"].rearrange("(a p) l -> p a l", p=128)
        q = queue or nc.sync
        ch = L // nch
        for lt in range(nch):
            q.dma_start(
                x_full[:, :, ch * lt:ch * (lt + 1)],
                src[:, :, ch * lt:ch * (lt + 1)],
            )
            if lt == 0 and mid is not None:
                mid()
        return x_full

    def pq_round(pp, m, wq, hc, t):
        ps = pp.tile([128, 512], F32, tag="pp", name=f"ppq{m}")
        nc.tensor.matmul(
            ps[:, 0:TQP],
            wq[:, :, 128 * hc:128 * (hc + 1)],
            xq8[m][:, :, TQP * t:TQP * (t + 1)],
            start=True, stop=True, perf_mode=PM.DoubleRow,
        )
        nc.vector.tensor_scalar_add(
            qT[m][:, hc, TQP * t:TQP * (t + 1)], ps[:, 0:TQP],
            bq_s[m][:, hc:hc + 1],
        )

    def proj_q(pp, m, wq):
        for hc in range(HC):
            for t in range(LQ // TQP):
                pq_round(pp, m, wq, hc, t)

    def k_round(pp, m, x_full, wk, hc, lt):
        ps = pp.tile([128, 512], F32, tag="pp", name=f"ppk{m}")
        nc.tensor.matmul(
            ps[:, 0:KT],
            wk[:, :, 128 * hc:128 * (hc + 1)],
            x_full[:, :, KT * lt:KT * (lt + 1)],
            start=True, stop=True, perf_mode=PM.DoubleRow,
        )
        nc.vector.tensor_copy(
            kT[m][:, hc, KT * lt:KT * (lt + 1)], ps[:, 0:KT]
        )

    def v_round(pp, m, x_full, wv, k):
        ps = pp.tile([128, 512], F32, tag="pp", name=f"ppv{m}")
        nc.tensor.matmul(
            ps[:],
            x_full[:, :, 128 * k:128 * (k + 1)],
            wv[:, :, :],
            start=True, stop=True, perf_mode=PM.DoubleRow,
        )
        vk = va[m][:, k // 2, k % 2, :, :]   # [128, NH, 65@80]
        nc.vector.tensor_copy(
            vk[:, :, 0:D], ps[:].rearrange("p (h d) -> p h d", d=D)
        )
        nc.vector.memset(vk[:, :, D:D + 1], 1.0)

    # ---------------- flat pipelined attention ----------------
    # unit = (tile ti, head-pair hp, chunk-pair cp); st slot s = 2a+j
    # (a = head parity, j = chunk parity) so head a's AV rhs is the
    # contiguous pt[:, 2a:2a+2, :].
    pending_fin = []

    def qk_emit(tiles, ti, hp, cp):
        qm, km, t = tiles[ti]
        toff = TQ * t
        st = st_pool.tile([128, 4, TQ], F32, padded_shape=[128, 4, 256],
                          tag="st", name="st")
        for j in range(2):
            k = 2 * cp + j
            for a in range(2):
                p0 = 64 * a
                nc.tensor.matmul(
                    st[:, 2 * a + j, :],
                    kT[km][p0:p0 + 64, hp, 128 * k:128 * (k + 1)],
                    qT[qm][p0:p0 + 64, hp, toff:toff + TQ],
                    start=True, stop=True,
                )
        return st

    def attention_flat(tiles, early_hooks, late_hooks):
        units = [(ti, hp, cp)
                 for ti in range(len(tiles))
                 for hp in range(NH // 2) for cp in range(NP)]
        ots = {}
        av_q = []   # AV work deferred by one pipeline slot
        sts = {0: qk_emit(tiles, *units[0])}
        prev_ti = 0

        def make_av(ti, hp, cp, pt):
            qm, km, t = tiles[ti]
            toff = TQ * t

            def av():
                if cp == 0:
                    ots[(ti, hp)] = ot_pool.tile(
                        [128, 2, TQ], F32, padded_shape=[128, 2, 256],
                        tag="ot", name="ot")
                ot = ots[(ti, hp)]
                for a in range(2):
                    h = 2 * hp + a
                    nc.tensor.matmul(
                        ot[0:65, a, :],
                        va[km][:, cp, :, h, :],
                        pt[:, 2 * a:2 * a + 2, :],
                        start=(cp == 0), stop=(cp == NP - 1),
                        perf_mode=PM.DoubleRow,
                    )
                if cp == NP - 1:
                    o_tmp = epi.tile([65, 2, TQ], F32, tag="o_tmp",
                                     name="o_tmp")
                    nc.vector.tensor_copy(o_tmp[:], ot[0:65, :, :])
                    # reciprocal_approx_fast writes fp32 into a bf16 tile via
                    # bitcast; the HIGH half of each fp32 word is its
                    # truncated-bf16 value, read below via [..., 1].
                    rrowb = epi.tile([65, 2, 2 * TQ], BF16, tag="rrowb",
                                     name="rrowb")
                    with nc.allow_low_precision(reason="softmax denom recip"):
                        nc.vector.reciprocal_approx_fast(
                            rrowb[:].bitcast(F32), o_tmp[:])
                    del ots[(ti, hp)]

                    def fin(qm=qm, hp=hp, toff=toff,
                            o_tmp=o_tmp, rrowb=rrowb):
                        pool, ptag = fp_pool["p"]
                        bc = pool.tile([64, 2, TQ], F32, tag=ptag, name="bc")
                        rvw = rrowb[D:D + 1, :, :].rearrange(
                            "p c (n two) -> p c n two", two=2)[:, :, :, 1]
                        nc.tensor.matmul(bc[:], ones_bc[D:D + 1, :],
                                         rvw, start=True, stop=True)
                        for a in range(2):
                            nc.vector.tensor_tensor(
                                ost[qm][64 * a:64 * a + 64, hp,
                                        toff:toff + TQ],
                                o_tmp[0:D, a, :], bc[:, a, :], ALU.mult,
                            )
                    pending_fin.append(fin)
            return av

        for i, (ti, hp, cp) in enumerate(units):
            if ti != prev_ti:
                # hooks only append fillers; fins/avs for the previous tile
                # drain naturally (fin pops at cp 0/1, av queue at cp<=2)
                # before the first filler slot at cp 3
                hook = late_hooks.get(ti)
                if hook:
                    hook()
                prev_ti = ti
            st = sts.pop(i)
            if ti > 0 and cp in DVE_CPS:
                # Schraudolph exp on the DVE; AV reads the low byte of each
                # fp32 word through a stride-4 fp8 bitcast view.
                ptf = ptp.tile([128, 4, TQ], F32, tag="ptf", name="ptf",
                               bufs=2)
                with nc.allow_low_precision(reason="schraudolph exp"):
                    nc.vector.tensor_scalar(
                        ptf[:], st[:, :, 0:TQ], EXPA8, EXPC8,
                        ALU.mult, ALU.add,
                    )
                pt = ptf[:].bitcast(FP8).rearrange(
                    "p s (n four) -> p s n four", four=4)[:, :, :, 0]
            else:
                ptb = ptp.tile([128, 4, TQ], FP8, tag="pt", name="pt", bufs=7)
                nc.scalar.activation(
                    ptb[:], st[:, :, 0:TQ], AF.Exp, bias=0.0,
                    scale=SCALE * LSC,
                )
                pt = ptb[:]
            last_ti = ti == len(tiles) - 1
            if ti == 0:
                fill(1)          # drain the projection backlog fast
            elif ti <= 2 and cp in (0, 2, 4, 6, 8):
                fill(1)
            elif ti >= 3 and cp in (3, 5, 7):
                fill(1)
            if i + 1 < len(units):
                nti = units[i + 1][0]
                if nti != ti:
                    hook = early_hooks.get(nti)
                    if hook:
                        # deferred AVs/fins may reference pools the hook
                        # closes — drain them first
                        while av_q:
                            av_q.pop(0)()
                        while pending_fin:
                            pending_fin.pop(0)()
                        hook()
                sts[i + 1] = qk_emit(tiles, *units[i + 1])
            av_q.append(make_av(ti, hp, cp, pt))
            if len(av_q) > (5 if ti == 0 else (1 if last_ti else 3)):
                av_q.pop(0)()
            if (cp in (0, 1, 5) or last_ti) and pending_fin:
                pending_fin.pop(0)()
        while av_q:
            av_q.pop(0)()
        while pending_fin:
            pending_fin.pop(0)()

    # ---------------- output proj + residual + LN ----------------
    def rsqrt_newton(out_ap, v_ap, scratch):
        """out = 1/sqrt(v) for v ~ 1; seed 1.5 - 0.5v + 1 Newton step."""
        r, s = scratch
        nc.vector.tensor_scalar(r[:], v_ap, -0.5, 1.5, ALU.mult, ALU.add)
        nc.vector.tensor_mul(s[:], r[:], r[:])
        nc.vector.tensor_mul(s[:], s[:], v_ap)
        nc.vector.tensor_scalar(s[:], s[:], -0.5, 1.5, ALU.mult, ALU.add)
        nc.vector.tensor_mul(out_ap, r[:], s[:])

    ystore = {}   # (m, t) -> (y_t, y2_t);  sstore: (m, t) -> (mu_sb, rs)
    sstore = {}

    def post_a_cc(post_pool, m, t, cc):
        toff = TQ * t
        if cc == 0:
            ystore[(m, t)] = (
                tmp.tile([128, CC, TQ], F32R, tag="y", name="y_t"),
                tmp.tile([128, CC, TQ], F32R, tag="y2", name="y2_t"),
            )
        y_t, y2_t = ystore[(m, t)]
        cps = post_pool.tile([128, 512], F32, tag="post", name="cps")
        for j in range(2):
            nc.tensor.matmul(
                cps[:, 0:TQ],
                woT[m][:, 2 * j:2 * j + 2, 128 * cc:128 * (cc + 1)],
                ost[m][:, 2 * j:2 * j + 2, toff:toff + TQ],
                start=(j == 0), stop=(j == 1),
                perf_mode=PM.DoubleRow,
            )
        nc.vector.scalar_tensor_tensor(
            y_t[:, cc, :], cps[:, 0:TQ], bo_s[m][:, cc:cc + 1],
            xq[m][:, cc, toff:toff + TQ], ALU.add, ALU.add,
        )
        nc.vector.tensor_mul(y2_t[:, cc, :], y_t[:, cc, :], y_t[:, cc, :])

    def post_b_stats(post_pool, m, t):
        y_t, y2_t = ystore[(m, t)]
        mu = post_pool.tile([128, 512], F32, tag="post", name="mu")
        for cc in range(CC):
            nc.tensor.matmul(
                mu[:, 0:TQ], ones_inv[:], y_t[:, cc, :],
                start=(cc == 0), stop=(cc == CC - 1),
            )
        for cc in range(CC):
            nc.tensor.matmul(
                mu[:, 256:256 + TQ], ones_inv[:], y2_t[:, cc, :],
                start=(cc == 0), stop=(cc == CC - 1),
            )
        mu_sb = tmp.tile([128, TQ], F32, tag="mu_sb", name="mu_sb")
        nc.vector.tensor_copy(mu_sb[:], mu[:, 0:TQ])
        x_t = tmp.tile([128, TQ], F32, tag="X", name="x_t")
        nc.vector.tensor_mul(x_t[:], mu_sb[:], mu_sb[:])
        nc.vector.tensor_sub(x_t[:], mu[:, 256:256 + TQ], x_t[:])
        nc.vector.tensor_scalar_add(x_t[:], x_t[:], EPS)
        rs = tmp.tile([128, TQ], F32, tag="rs", name="rs")
        sc = tmp.tile([128, TQ], F32, tag="sc", name="sc")
        rsqrt_newton(rs[:], x_t[:], (rs, sc))
        sstore[(m, t)] = (mu_sb, rs)

    def post_b_apply(m, t, cc):
        toff = TQ * t
        y_t, _ = ystore[(m, t)]
        mu_sb, rs = sstore[(m, t)]
        nc.vector.tensor_sub(y_t[:, cc, :], y_t[:, cc, :], mu_sb[:])
        nc.vector.tensor_mul(y_t[:, cc, :], y_t[:, cc, :], rs[:])
        nc.vector.tensor_scalar(
            msb[m][:, cc, toff:toff + TQ], y_t[:, cc, :],
            lng_s[m][:, cc:cc + 1], lnb_s[m][:, cc:cc + 1],
            ALU.mult, ALU.add,
        )

    def fuse_cc(post_pool, t, cc):
        toff = TQ * t
        fp = post_pool.tile([128, 512], F32, tag="post", name="fp")
        for j in range(HC):
            src = msb[1] if j < CC else msb[2]
            nc.tensor.matmul(
                fp[:, 0:TQ],
                wfT[:, j, 128 * cc:128 * (cc + 1)],
                src[:, j % CC, toff:toff + TQ],
                start=(j == 0), stop=(j == HC - 1),
            )
        f_sb = tmp.tile([128, TQ], F32, tag="f", name="f_sb")
        nc.scalar.activation(
            f_sb[:], fp[:, 0:TQ], AF.Relu,
            bias=bnb_s[:, cc:cc + 1], scale=bnw_s[:, cc:cc + 1],
        )
        nc.sync.dma_start(
            y_out.rearrange("(a p) l -> p a l", p=128)[
                :, cc, toff:toff + TQ
            ],
            f_sb[:],
        )

    def post_all(post_pool, m, t):
        for cc in range(CC):
            post_a_cc(post_pool, m, t, cc)
        post_b_stats(post_pool, m, t)
        for cc in range(CC):
            post_b_apply(m, t, cc)

    # ---------------- emission schedule ----------------
    pc2 = ExitStack()
    pp2 = pc2.enter_context(tc.tile_pool(name="pp", bufs=2, space="PSUM"))
    wq2, wk2, wv2 = open_w(pc2, 2, dma=False)
    pc1 = ExitStack()
    wq1, wk1, wv1 = open_w(pc1, 1, dma=False)
    # DMA priority: the first attention group needs wq2+x2q (qT2 hc0) and
    # wk1 + x1f chunk 0 (kT1 hc0 keys 0-383); everything else trails.
    dma_w(wq2, "wq", 2)
    nc.sync.dma_start(
        xq8[2][:], ins["x2q8"].rearrange("(a p) l -> p a l", p=128))
    dma_w(wk1, "wk", 1)

    def mid():
        dma_w(wv1, "wv", 1)
        dma_w(wq1, "wq", 1)
        nc.sync.dma_start(
            xq8[1][:], ins["x1q8"].rearrange("(a p) l -> p a l", p=128))
        nc.sync.dma_start(
            xq[2][:], ins["x2q"].rearrange("(a p) l -> p a l", p=128))
        nc.sync.dma_start(
            xq[1][:], ins["x1q"].rearrange("(a p) l -> p a l", p=128))
    x1 = open_x(pc1, 1, mid=mid, nch=6)
    for w, wn in ((wk2, "wk"), (wv2, "wv")):
        dma_w(w, wn, 2)
    pc3 = ExitStack()
    x2 = open_x(pc3, 2, nch=6)

    # minimal pre-attention PE work; the rest of modal-1's projections are
    # fillers drained one per group during tile 0 (ordered by first use:
    # kT1-hc0 chunks gate hp0's QK sweep, va1 chunks gate its AVs, then
    # qT2-hc1..3 / kT1-hc1..3 ahead of hp1..3, then modal-2 k/v for dir 1->2)
    pq_round(pp2, 2, wq2, 0, 0)
    for lt in range(L // KT):
        k_round(pp2, 1, x1, wk1, 0, lt)
    for k in range(10):
        v_round(pp2, 1, x1, wv1, k)

    def K1(hc, lt):
        return lambda: k_round(pp2, 1, x1, wk1, hc, lt)

    def V1(k):
        return lambda: v_round(pp2, 1, x1, wv1, k)

    def Q2(hc, t):
        return lambda: pq_round(pp2, 2, wq2, hc, t)

    # ordered by first-use (1 fill/group during tile 0: qk of unit u has u
    # fills before it, av of chunk-pair cp has cp+6 given the defer-5 queue)
    pro = [V1(10), V1(11), Q2(1, 0), K1(1, 0), V1(12), V1(13), K1(1, 1),
           V1(14), K1(1, 2), V1(15), V1(16), K1(1, 3), V1(17), K1(1, 4),
           K1(1, 5), Q2(2, 0), K1(2, 0), K1(2, 1), K1(2, 2), K1(2, 3),
           K1(2, 4), K1(2, 5), Q2(3, 0), K1(3, 0), K1(3, 1), K1(3, 2),
           K1(3, 3), K1(3, 4), K1(3, 5),
           Q2(0, 1), Q2(1, 1), Q2(2, 1), Q2(3, 1)]
    fillers.extend(pro)

    # modal-1 Q projection + modal 2 k/v rounds: fillers for tiles 1-2
    fillers.append(lambda: pq_round(pp2, 1, wq1, 0, 0))
    kv2 = []
    for hc in range(HC):
        for lt in range(L // KT):
            kv2.append(lambda hc=hc, lt=lt: k_round(pp2, 2, x2, wk2, hc, lt))
    vstart = len(kv2)
    for k in range(NK):
        kv2.append(lambda k=k: v_round(pp2, 2, x2, wv2, k))
    mixed = []
    ki, vi = 0, vstart
    while ki < vstart or vi < len(kv2):
        if ki < vstart:
            mixed.append(kv2[ki]); ki += 1
            if ki < vstart:
                mixed.append(kv2[ki]); ki += 1
        if vi < len(kv2):
            mixed.append(kv2[vi]); vi += 1
    for i, t in enumerate([(0, 1), (1, 0), (1, 1), (2, 0), (2, 1), (3, 0),
                           (3, 1)]):
        mixed.insert(5 * i + 4,
                     lambda hc=t[0], t_=t[1]: pq_round(pp2, 1, wq1, hc, t_))
    fillers.extend(mixed)

    holder = {}
    fp_pool = {}
    y2t = {}
    y1t = {}

    def early_dir():
        # all modal-2 projections must be emitted before dir 1->2 reads them
        fill(len(fillers))
        pc3.close()
        pc1.close()
        pc2.close()
        holder["post"] = ctx.enter_context(
            tc.tile_pool(name="post", bufs=2, space="PSUM"))
        fp_pool["p"] = (holder["post"], "post")

    def post_fillers(m, t):
        pool = holder["post"]
        for cc in range(CC):
            fillers.append(lambda cc=cc: post_a_cc(pool, m, t, cc))
        fillers.append(lambda: post_b_stats(pool, m, t))
        for cc in range(CC):
            fillers.append(lambda cc=cc: post_b_apply(m, t, cc))

    def late_dir():
        for t in range(NT):
            post_fillers(2, t)

    def late_t1():
        pool = holder["post"]
        post_fillers(1, 0)
        for cc in range(CC):
            fillers.append(lambda cc=cc: fuse_cc(pool, 0, cc))

    def late_t2():
        pool = holder["post"]
        post_fillers(1, 1)
        for cc in range(CC):
            fillers.append(lambda cc=cc: fuse_cc(pool, 1, cc))

    # fin/bc before the post pool opens parks bc tiles in pp2's ring instead
    fp_pool["p"] = (pp2, "pp")

    tiles = [(2, 1, 0), (2, 1, 1), (2, 1, 2), (1, 2, 0), (1, 2, 1), (1, 2, 2)]
    attention_flat(tiles, {3: early_dir},
                   {3: late_dir, 4: late_t1, 5: late_t2})
    fill(len(fillers))
    pool = holder["post"]
    post_all(pool, 1, 2)
    for cc in range(CC):
        fuse_cc(pool, 2, cc)


def host_prep(inputs):
    """Precompute transposed weights / folded biases; slice per-core inputs."""
    import ml_dtypes
    bf = lambda a: np.ascontiguousarray(
        np.asarray(a, np.float32), dtype=ml_dtypes.bfloat16)
    pvals = {
        "bq1": inputs["bq1"] * WSC, "bq2": inputs["bq2"] * WSC,
        "bo1p": inputs["bo1"] + inputs["wo1"] @ inputs["bv1"],
        "bo2p": inputs["bo2"] + inputs["wo2"] @ inputs["bv2"],
        "ln1g": inputs["ln1_g"], "ln1b": inputs["ln1_b"],
        "ln2g": inputs["ln2_g"], "ln2b": inputs["ln2_b"],
    }
    bnw = inputs["bn_g"] / np.sqrt(inputs["bn_var"] + EPS)
    pvals["bnw"] = bnw
    pvals["bnb"] = (inputs["bf"] - inputs["bn_mean"]) * bnw + inputs["bn_b"]
    packed = np.zeros((128, NPARAM_COLS), np.float32)
    for nm, (off, ch) in _PARAM_SLOTS.items():
        packed[:, off:off + ch] = np.asarray(pvals[nm], np.float32).reshape(
            ch, 128).T

    f8 = lambda a: np.ascontiguousarray(
        np.asarray(a, np.float32), dtype=ml_dtypes.float8_e4m3)
    f85 = lambda a: np.ascontiguousarray(
        np.asarray(a, np.float32), dtype=ml_dtypes.float8_e5m2)
    shared = {
        "params": packed,
        "wq1T": f8(inputs["wq1"].T * WSC), "wk1T": f8(inputs["wk1"].T * WSC),
        "wv1T": f8(inputs["wv1"].T * WSC), "wq2T": f8(inputs["wq2"].T * WSC),
        "wk2T": f8(inputs["wk2"].T * WSC), "wv2T": f8(inputs["wv2"].T * WSC),
        "wo1T": f85(inputs["wo1"].T), "wo2T": f85(inputs["wo2"].T),
        "wfT": bf(inputs["wf"].T),
    }
    x1 = np.asarray(inputs["modal1_feat"], np.float32).reshape(B, C, L)
    x2 = np.asarray(inputs["modal2_feat"], np.float32).reshape(B, C, L)
    in_maps = []
    for core in range(NCORES):
        b, q = core // 4, core % 4
        m = dict(shared)
        m["x1f"] = f8(x1[b])
        m["x2f"] = f8(x2[b])
        m["x1q"] = bf(x1[b][:, LQ * q:LQ * (q + 1)])
        m["x2q"] = bf(x2[b][:, LQ * q:LQ * (q + 1)])
        m["x1q8"] = f8(x1[b][:, LQ * q:LQ * (q + 1)])
        m["x2q8"] = f8(x2[b][:, LQ * q:LQ * (q + 1)])
        in_maps.append(m)
    return in_maps


_IN_SPECS = [
    ("x1f", (C, L)), ("x2f", (C, L)), ("x1q", (C, LQ)), ("x2q", (C, LQ)),
    ("x1q8", (C, LQ)), ("x2q8", (C, LQ)),
    ("wq1T", (C, HID)), ("wk1T", (C, HID)), ("wv1T", (C, HID)),
    ("wq2T", (C, HID)), ("wk2T", (C, HID)), ("wv2T", (C, HID)),
    ("wo1T", (HID, C)), ("wo2T", (HID, C)), ("wfT", (HID, C)),
    ("params", (128, NPARAM_COLS)),
]

_BF16_INS = {"x1q", "x2q", "wfT"}
_FP8_INS = {"x1f", "x2f", "x1q8", "x2q8", "wq1T", "wk1T", "wv1T",
            "wq2T", "wk2T", "wv2T"}
_FP8E5_INS = {"wo1T", "wo2T"}


def build_program():
    nc = bacc.Bacc("TRN2", target_bir_lowering=False, debug=False)
    F8E5 = mybir.dt.float8e5
    ins = {
        name: nc.dram_tensor(
            name, list(shape),
            F8E5 if name in _FP8E5_INS else (
                FP8 if name in _FP8_INS else (
                    BF16 if name in _BF16_INS else F32)),
            kind="ExternalInput",
        ).ap()
        for name, shape in _IN_SPECS
    }
    outs = {"y": nc.dram_tensor("y", [C, LQ], F32, kind="ExternalOutput").ap()}
    with tile.TileContext(nc) as tc:
        core_kernel(tc, outs, ins)
    nc.compile()
    return nc


def _install_ntff_hook():
    """Provide antenv.axon_hooks (absent in this image) so trace=True works."""
    import sys, types
    if "antenv.axon_hooks" in sys.modules:
        return
    try:
        from trn_agent_boot.trn_boot import _ntff_profile_via_ctypes
        hook = _ntff_profile_via_ctypes("/opt/axon/libaxon_pjrt.so")
    except Exception:
        hook = None
    mod = types.ModuleType("antenv.axon_hooks")
    state = {"hook": hook}
    mod.set_axon_ntff_profile_hook = lambda h: state.__setitem__("hook", h)
    mod.get_axon_ntff_profile_hook = lambda: state["hook"]
    sys.modules["antenv.axon_hooks"] = mod


def kernel(**inputs) -> np.ndarray:
    global LAST_EXEC_NS, LAST_RESULTS
    from concourse.bass_utils import run_bass_kernel_spmd

    in_maps = host_prep(inputs)
    nc = build_program()
    trace = bool(int(os.environ.get("MMPAF_TRACE", "0")))
    if trace:
        _install_ntff_hook()
    res = run_bass_kernel_spmd(
        nc, in_maps, core_ids=list(range(NCORES)), trace=trace
    )
    LAST_EXEC_NS = res.exec_time_ns
    LAST_RESULTS = res
    out = np.empty((B, C, L), np.float32)
    for core in range(NCORES):
        b, q = core // 4, core % 4
        out[b, :, LQ * q:LQ * (q + 1)] = res.results[core]["y"]
    return out.reshape(B, C, H, W)


# revision 4
# speedup vs baseline: 1.0246x; 1.0246x over previous
"""MultiModalPyramidAttentionFusion — Trainium2 Bass/Tile kernel.

Full inputs in, full output out. 8-way SPMD over (batch b) x (query-pixel
quarter q); each core computes the fused output for its 576 query pixels.
K/V projections (full 2304-pixel image) are replicated across the 4 cores
of a batch element — no collectives.

v2 attention core (vs v1): query tiles are 192 wide so QK logit chunks
pack two-to-a-PSUM-bank; the attention unit is a (head-pair, key-chunk-
pair) group. Per group: 4 QK matmuls (N=192) alternate head parity so the
two heads' K=64 matmuls run row-tile-CONCURRENT in the PE array (base
partitions 0/64), one fused EXP over all 4 chunks (FD=768, 3D AP — ACT
cost 260+FD/1.2 so fusing 4 chunks amortizes the fixed cost), and one
fp8-e4m3 DoubleRow AV matmul per head that contracts both 128-key chunks
in a single N=192 stream (2 fp8 MACs/cell/cycle). EXP writes fp8 directly;
V is stored fp8 with a ones column so softmax denominators fall out of the
same matmul. Both heads' AV accumulators pack into ONE PSUM bank
([128,2,192@256]), so the whole steady state fits 8 banks:
st 2x2 + ot 2x1 + filler 2x1.

Precision: projection path in bf16; P and V in fp8e4 (P in [0.67,1.5] so
e4m3 rel err ~3%, washes out over 2304-key softmax averaging; V errors
average the same way). K-projection bias dropped (cancels in softmax).
LN stats fp32; rstd via DVE Newton rsqrt.

Independent matmul work (modal-2 K/V projections, output projection / LN
stats / fusion) drains as PE filler between groups, same as v1.
"""

import os
from contextlib import ExitStack

import numpy as np

import concourse.bass as bass
import concourse.mybir as mybir
import concourse.tile as tile
from concourse import bacc
from concourse._compat import with_exitstack

F32 = mybir.dt.float32
F32R = mybir.dt.float32r
BF16 = mybir.dt.bfloat16
FP8 = mybir.dt.float8e4
AF = mybir.ActivationFunctionType
ALU = mybir.AluOpType
PM = mybir.MatmulPerfMode

B, C, H, W = 2, 256, 48, 48
L = H * W            # 2304
HID, NH, D = 512, 8, 64
EPS = 1e-5
SCALE = D ** -0.5    # 1/8

NCORES = 8
# Schraudolph exp for DVE offload: one tensor_scalar computes
# u = x*(SCALE*log2e*8) + (2^23 + 56 - 0.347) in fp32; the +2^23 forces
# round-to-int in the mantissa and the LOW 8 bits of the fp32 word are
# exactly the fp8e4m3 bits of 2^z (max rel err ~9%, washes out in the
# 2304-key softmax average).
WSC = 32.0           # host scale on wq/wk/wv so fp8e4 stays in normal range
LSC = 1.0 / (WSC * WSC)   # logit descale, folded into the EXP scale
EXPA8 = float(np.log2(np.e) * LSC)    # SCALE * log2e * 8 * LSC
EXPC8 = 8388663.653
# chunk-pair indices whose exp runs on the DVE instead of ACT (per tile>=1)
DVE_CPS = frozenset(
    int(x) for x in os.environ.get("MMPAF_DVE_CPS", "3").split(",") if x)
LQ = L // 4          # 576 query pixels per core
NT = 3               # query tiles per core per modal
TQ = LQ // NT        # 192-wide query tiles
NK = L // 128        # 18 key chunks
NP = NK // 2         # 9 key-chunk pairs
CC = C // 128        # 2 channel chunks
HC = HID // 128      # 4 hidden chunks
KT = 384             # free-tile for k projection (L = 6*384)
TQP = 288            # projection round width for q (LQ = 2*288)

# packed per-partition parameter layout: name -> (col offset, chunks)
_PARAM_SLOTS = {}
_off = 0
for _nm, _ch in [("bq1", 4), ("bq2", 4),
                 ("bo1p", 2), ("bo2p", 2), ("ln1g", 2), ("ln1b", 2),
                 ("ln2g", 2), ("ln2b", 2), ("bnw", 2), ("bnb", 2)]:
    _PARAM_SLOTS[_nm] = (_off, _ch)
    _off += _ch
NPARAM_COLS = _off  # 24

LAST_EXEC_NS = None
LAST_RESULTS = None


@with_exitstack
def core_kernel(ctx: ExitStack, tc: tile.TileContext, outs, ins):
    nc = tc.nc
    y_out = outs["y"]  # [256, 576]

    # ---------------- pools ----------------
    consts = ctx.enter_context(tc.tile_pool(name="consts", bufs=1))
    big = ctx.enter_context(tc.tile_pool(name="big", bufs=1))
    ptp = ctx.enter_context(tc.tile_pool(name="ptp", bufs=4))
    epi = ctx.enter_context(tc.tile_pool(name="epi", bufs=2))
    tmp = ctx.enter_context(tc.tile_pool(name="tmp", bufs=2))

    st_pool = ctx.enter_context(tc.tile_pool(name="st", bufs=2, space="PSUM"))
    ot_pool = ctx.enter_context(tc.tile_pool(name="ot", bufs=2, space="PSUM"))

    # ---------------- params (single packed DMA) ----------------
    params = consts.tile([128, NPARAM_COLS], F32)
    nc.gpsimd.dma_start(params[:], ins["params"][:])

    def prm(name):
        off, ch = _PARAM_SLOTS[name]
        return params[:, off:off + ch]

    bq_s = {1: prm("bq1"), 2: prm("bq2")}
    bo_s = {1: prm("bo1p"), 2: prm("bo2p")}
    lng_s = {1: prm("ln1g"), 2: prm("ln2g")}
    lnb_s = {1: prm("ln1b"), 2: prm("ln2b")}
    bnw_s, bnb_s = prm("bnw"), prm("bnb")

    ones_f32 = consts.tile([128, 128], F32)
    nc.vector.memset(ones_f32[:], 1.0 / C)
    ones_inv = consts.tile([128, 128], F32R)
    nc.vector.tensor_copy(ones_inv[:], ones_f32[:])
    # 1/WSC here un-scales the x32 V values through the denominator broadcast
    ones_bc = consts.tile([128, 64], BF16)
    nc.vector.memset(ones_bc[:], 1.0 / WSC)

    # ---------------- big SBUF tensors ----------------
    qT = {m: big.tile([128, HC, LQ], BF16, tag=f"qT{m}", name=f"qT{m}")
          for m in (1, 2)}
    kT = {m: big.tile([128, HC, L], BF16, tag=f"kT{m}", name=f"kT{m}")
          for m in (1, 2)}
    # V (+ones col) in fp8, keyed [chunk-pair, chunk-parity, head, 65@80]
    va = {m: big.tile([128, NP, 2, NH, 65], FP8,
                      padded_shape=[128, NP, 2, NH, 80],
                      tag=f"va{m}", name=f"va{m}")
          for m in (1, 2)}
    FP8E5 = mybir.dt.float8e5
    ost = {m: big.tile([128, HC, LQ], FP8E5, tag=f"ost{m}", name=f"ost{m}")
           for m in (1, 2)}
    msb = {m: big.tile([128, CC, LQ], BF16, tag=f"m{m}", name=f"msb{m}")
           for m in (1, 2)}
    xq = {}
    xq8 = {}
    for m in (1, 2):
        xq[m] = big.tile([128, CC, LQ], BF16, tag=f"xq{m}", name=f"xq{m}")
        xq8[m] = big.tile([128, CC, LQ], FP8, tag=f"xq8{m}", name=f"xq8{m}")

    woT = {}
    for m in (1, 2):
        woT[m] = big.tile([128, HC, C], FP8E5, tag=f"woT{m}", name=f"woT{m}")
        nc.gpsimd.dma_start(
            woT[m][:], ins[f"wo{m}T"].rearrange("(a p) c -> p a c", p=128)
        )
    wfT = big.tile([128, HC, C], BF16, tag="wfT")
    nc.gpsimd.dma_start(wfT[:], ins["wfT"].rearrange("(a p) c -> p a c", p=128))

    # ---------------- filler machinery ----------------
    fillers = []      # closures of independent PE work, drained in attention

    def fill(n):
        for _ in range(n):
            if not fillers:
                return
            fillers.pop(0)()

    # ---------------- projections ----------------
    def open_w(ctx2, m, dma=True):
        wp = ctx2.enter_context(tc.tile_pool(name=f"wp{m}", bufs=1))
        ws = []
        for wn in ("wq", "wk", "wv"):
            w = wp.tile([128, CC, HID], FP8, tag=wn, name=f"{wn}{m}")
            if dma:
                dma_w(w, wn, m)
            ws.append(w)
        return ws

    def dma_w(w, wn, m):
        nc.sync.dma_start(
            w[:], ins[f"{wn}{m}T"].rearrange("(a p) h -> p a h", p=128)
        )

    def open_x(ctx2, m, mid=None, queue=None, nch=2):
        xf = ctx2.enter_context(tc.tile_pool(name=f"xf{m}", bufs=1))
        x_full = xf.tile([128, CC, L], FP8, tag="xfull", name=f"xfull{m}")
        src = ins[f"x{m}f"].rearrange("(a p) l -> p a l", p=128)
        q = queue or nc.sync
        ch = L // nch
        for lt in range(nch):
            q.dma_start(
                x_full[:, :, ch * lt:ch * (lt + 1)],
                src[:, :, ch * lt:ch * (lt + 1)],
            )
            if lt == 0 and mid is not None:
                mid()
        return x_full

    def pq_round(pp, m, wq, hc, t):
        ps = pp.tile([128, 512], F32, tag="pp", name=f"ppq{m}")
        nc.tensor.matmul(
            ps[:, 0:TQP],
            wq[:, :, 128 * hc:128 * (hc + 1)],
            xq8[m][:, :, TQP * t:TQP * (t + 1)],
            start=True, stop=True, perf_mode=PM.DoubleRow,
        )
        nc.vector.tensor_scalar_add(
            qT[m][:, hc, TQP * t:TQP * (t + 1)], ps[:, 0:TQP],
            bq_s[m][:, hc:hc + 1],
        )

    def proj_q(pp, m, wq):
        for hc in range(HC):
            for t in range(LQ // TQP):
                pq_round(pp, m, wq, hc, t)

    def k_round(pp, m, x_full, wk, hc, lt):
        ps = pp.tile([128, 512], F32, tag="pp", name=f"ppk{m}")
        nc.tensor.matmul(
            ps[:, 0:KT],
            wk[:, :, 128 * hc:128 * (hc + 1)],
            x_full[:, :, KT * lt:KT * (lt + 1)],
            start=True, stop=True, perf_mode=PM.DoubleRow,
        )
        nc.vector.tensor_copy(
            kT[m][:, hc, KT * lt:KT * (lt + 1)], ps[:, 0:KT]
        )

    def v_round(pp, m, x_full, wv, k):
        ps = pp.tile([128, 512], F32, tag="pp", name=f"ppv{m}")
        nc.tensor.matmul(
            ps[:],
            x_full[:, :, 128 * k:128 * (k + 1)],
            wv[:, :, :],
            start=True, stop=True, perf_mode=PM.DoubleRow,
        )
        vk = va[m][:, k // 2, k % 2, :, :]   # [128, NH, 65@80]
        nc.vector.tensor_copy(
            vk[:, :, 0:D], ps[:].rearrange("p (h d) -> p h d", d=D)
        )
        nc.vector.memset(vk[:, :, D:D + 1], 1.0)

    # ---------------- flat pipelined attention ----------------
    # unit = (tile ti, head-pair hp, chunk-pair cp); st slot s = 2a+j
    # (a = head parity, j = chunk parity) so head a's AV rhs is the
    # contiguous pt[:, 2a:2a+2, :].
    pending_fin = []

    def qk_emit(tiles, ti, hp, cp):
        qm, km, t = tiles[ti]
        toff = TQ * t
        st = st_pool.tile([128, 4, TQ], F32, padded_shape=[128, 4, 256],
                          tag="st", name="st")
        for j in range(2):
            k = 2 * cp + j
            for a in range(2):
                p0 = 64 * a
                nc.tensor.matmul(
                    st[:, 2 * a + j, :],
                    kT[km][p0:p0 + 64, hp, 128 * k:128 * (k + 1)],
                    qT[qm][p0:p0 + 64, hp, toff:toff + TQ],
                    start=True, stop=True,
                )
        return st

    def attention_flat(tiles, early_hooks, late_hooks):
        units = [(ti, hp, cp)
                 for ti in range(len(tiles))
                 for hp in range(NH // 2) for cp in range(NP)]
        ots = {}
        av_q = []   # AV work deferred by one pipeline slot
        sts = {0: qk_emit(tiles, *units[0])}
        prev_ti = 0

        def make_av(ti, hp, cp, pt):
            qm, km, t = tiles[ti]
            toff = TQ * t

            def av():
                if cp == 0:
                    ots[(ti, hp)] = ot_pool.tile(
                        [128, 2, TQ], F32, padded_shape=[128, 2, 256],
                        tag="ot", name="ot")
                ot = ots[(ti, hp)]
                for a in range(2):
                    h = 2 * hp + a
                    nc.tensor.matmul(
                        ot[0:65, a, :],
                        va[km][:, cp, :, h, :],
                        pt[:, 2 * a:2 * a + 2, :],
                        start=(cp == 0), stop=(cp == NP - 1),
                        perf_mode=PM.DoubleRow,
                    )
                if cp == NP - 1:
                    o_tmp = epi.tile([65, 2, TQ], F32, tag="o_tmp",
                                     name="o_tmp")
                    nc.vector.tensor_copy(o_tmp[:], ot[0:65, :, :])
                    # reciprocal_approx_fast writes fp32 into a bf16 tile via
                    # bitcast; the HIGH half of each fp32 word is its
                    # truncated-bf16 value, read below via [..., 1].
                    rrowb = epi.tile([65, 2, 2 * TQ], BF16, tag="rrowb",
                                     name="rrowb")
                    with nc.allow_low_precision(reason="softmax denom recip"):
                        nc.vector.reciprocal_approx_fast(
                            rrowb[:].bitcast(F32), o_tmp[:])
                    del ots[(ti, hp)]

                    def fin(qm=qm, hp=hp, toff=toff,
                            o_tmp=o_tmp, rrowb=rrowb):
                        pool, ptag = fp_pool["p"]
                        bc = pool.tile([64, 2, TQ], F32, tag=ptag, name="bc")
                        rvw = rrowb[D:D + 1, :, :].rearrange(
                            "p c (n two) -> p c n two", two=2)[:, :, :, 1]
                        nc.tensor.matmul(bc[:], ones_bc[D:D + 1, :],
                                         rvw, start=True, stop=True)
                        for a in range(2):
                            nc.vector.tensor_tensor(
                                ost[qm][64 * a:64 * a + 64, hp,
                                        toff:toff + TQ],
                                o_tmp[0:D, a, :], bc[:, a, :], ALU.mult,
                            )
                    pending_fin.append(fin)
            return av

        for i, (ti, hp, cp) in enumerate(units):
            if ti != prev_ti:
                # hooks only append fillers; fins/avs for the previous tile
                # drain naturally (fin pops at cp 0/1, av queue at cp<=2)
                # before the first filler slot at cp 3
                hook = late_hooks.get(ti)
                if hook:
                    hook()
                prev_ti = ti
            st = sts.pop(i)
            if ti > 0 and cp in DVE_CPS:
                # Schraudolph exp on the DVE; AV reads the low byte of each
                # fp32 word through a stride-4 fp8 bitcast view.
                ptf = ptp.tile([128, 4, TQ], F32, tag="ptf", name="ptf",
                               bufs=2)
                with nc.allow_low_precision(reason="schraudolph exp"):
                    nc.vector.tensor_scalar(
                        ptf[:], st[:, :, 0:TQ], EXPA8, EXPC8,
                        ALU.mult, ALU.add,
                    )
                pt = ptf[:].bitcast(FP8).rearrange(
                    "p s (n four) -> p s n four", four=4)[:, :, :, 0]
            else:
                ptb = ptp.tile([128, 4, TQ], FP8, tag="pt", name="pt", bufs=7)
                nc.scalar.activation(
                    ptb[:], st[:, :, 0:TQ], AF.Exp, bias=0.0,
                    scale=SCALE * LSC,
                )
                pt = ptb[:]
            last_ti = ti == len(tiles) - 1
            if ti == 0:
                fill(1)          # drain the projection backlog fast
            elif ti <= 2 and cp in (0, 2, 4, 6, 8):
                fill(1)
            elif ti >= 3 and cp in (3, 5, 7):
                fill(1)
            if i + 1 < len(units):
                nti = units[i + 1][0]
                if nti != ti:
                    hook = early_hooks.get(nti)
                    if hook:
                        # deferred AVs/fins may reference pools the hook
                        # closes — drain them first
                        while av_q:
                            av_q.pop(0)()
                        while pending_fin:
                            pending_fin.pop(0)()
                        hook()
                sts[i + 1] = qk_emit(tiles, *units[i + 1])
            av_q.append(make_av(ti, hp, cp, pt))
            if len(av_q) > (5 if ti == 0 else (1 if last_ti else 3)):
                av_q.pop(0)()
            if (cp in (0, 1, 5) or last_ti) and pending_fin:
                pending_fin.pop(0)()
        while av_q:
            av_q.pop(0)()
        while pending_fin:
            pending_fin.pop(0)()

    # ---------------- output proj + residual + LN ----------------
    def rsqrt_newton(out_ap, v_ap, scratch):
        """out = 1/sqrt(v) for v ~ 1; seed 1.5 - 0.5v + 1 Newton step."""
        r, s = scratch
        nc.vector.tensor_scalar(r[:], v_ap, -0.5, 1.5, ALU.mult, ALU.add)
        nc.vector.tensor_mul(s[:], r[:], r[:])
        nc.vector.tensor_mul(s[:], s[:], v_ap)
        nc.vector.tensor_scalar(s[:], s[:], -0.5, 1.5, ALU.mult, ALU.add)
        nc.vector.tensor_mul(out_ap, r[:], s[:])

    ystore = {}   # (m, t) -> (y_t, y2_t);  sstore: (m, t) -> (mu_sb, rs)
    sstore = {}

    def post_a_cc(post_pool, m, t, cc):
        toff = TQ * t
        if cc == 0:
            ystore[(m, t)] = (
                tmp.tile([128, CC, TQ], F32R, tag="y", name="y_t"),
                tmp.tile([128, CC, TQ], F32R, tag="y2", name="y2_t"),
            )
        y_t, y2_t = ystore[(m, t)]
        cps = post_pool.tile([128, 512], F32, tag="post", name="cps")
        for j in range(2):
            nc.tensor.matmul(
                cps[:, 0:TQ],
                woT[m][:, 2 * j:2 * j + 2, 128 * cc:128 * (cc + 1)],
                ost[m][:, 2 * j:2 * j + 2, toff:toff + TQ],
                start=(j == 0), stop=(j == 1),
                perf_mode=PM.DoubleRow,
            )
        nc.vector.scalar_tensor_tensor(
            y_t[:, cc, :], cps[:, 0:TQ], bo_s[m][:, cc:cc + 1],
            xq[m][:, cc, toff:toff + TQ], ALU.add, ALU.add,
        )
        nc.vector.tensor_mul(y2_t[:, cc, :], y_t[:, cc, :], y_t[:, cc, :])

    def post_b_stats(post_pool, m, t):
        y_t, y2_t = ystore[(m, t)]
        mu = post_pool.tile([128, 512], F32, tag="post", name="mu")
        for cc in range(CC):
            nc.tensor.matmul(
                mu[:, 0:TQ], ones_inv[:], y_t[:, cc, :],
                start=(cc == 0), stop=(cc == CC - 1),
            )
        for cc in range(CC):
            nc.tensor.matmul(
                mu[:, 256:256 + TQ], ones_inv[:], y2_t[:, cc, :],
                start=(cc == 0), stop=(cc == CC - 1),
            )
        mu_sb = tmp.tile([128, TQ], F32, tag="mu_sb", name="mu_sb")
        nc.vector.tensor_copy(mu_sb[:], mu[:, 0:TQ])
        x_t = tmp.tile([128, TQ], F32, tag="X", name="x_t")
        nc.vector.tensor_mul(x_t[:], mu_sb[:], mu_sb[:])
        nc.vector.tensor_sub(x_t[:], mu[:, 256:256 + TQ], x_t[:])
        nc.vector.tensor_scalar_add(x_t[:], x_t[:], EPS)
        rs = tmp.tile([128, TQ], F32, tag="rs", name="rs")
        sc = tmp.tile([128, TQ], F32, tag="sc", name="sc")
        rsqrt_newton(rs[:], x_t[:], (rs, sc))
        sstore[(m, t)] = (mu_sb, rs)

    def post_b_apply(m, t, cc):
        toff = TQ * t
        y_t, _ = ystore[(m, t)]
        mu_sb, rs = sstore[(m, t)]
        nc.vector.tensor_sub(y_t[:, cc, :], y_t[:, cc, :], mu_sb[:])
        nc.vector.tensor_mul(y_t[:, cc, :], y_t[:, cc, :], rs[:])
        nc.vector.tensor_scalar(
            msb[m][:, cc, toff:toff + TQ], y_t[:, cc, :],
            lng_s[m][:, cc:cc + 1], lnb_s[m][:, cc:cc + 1],
            ALU.mult, ALU.add,
        )

    def fuse_cc(post_pool, t, cc):
        toff = TQ * t
        fp = post_pool.tile([128, 512], F32, tag="post", name="fp")
        for j in range(HC):
            src = msb[1] if j < CC else msb[2]
            nc.tensor.matmul(
                fp[:, 0:TQ],
                wfT[:, j, 128 * cc:128 * (cc + 1)],
                src[:, j % CC, toff:toff + TQ],
                start=(j == 0), stop=(j == HC - 1),
            )
        f_sb = tmp.tile([128, TQ], F32, tag="f", name="f_sb")
        nc.scalar.activation(
            f_sb[:], fp[:, 0:TQ], AF.Relu,
            bias=bnb_s[:, cc:cc + 1], scale=bnw_s[:, cc:cc + 1],
        )
        nc.sync.dma_start(
            y_out.rearrange("(a p) l -> p a l", p=128)[
                :, cc, toff:toff + TQ
            ],
            f_sb[:],
        )

    def post_all(post_pool, m, t):
        for cc in range(CC):
            post_a_cc(post_pool, m, t, cc)
        post_b_stats(post_pool, m, t)
        for cc in range(CC):
            post_b_apply(m, t, cc)

    # ---------------- emission schedule ----------------
    pc2 = ExitStack()
    pp2 = pc2.enter_context(tc.tile_pool(name="pp", bufs=2, space="PSUM"))
    wq2, wk2, wv2 = open_w(pc2, 2, dma=False)
    pc1 = ExitStack()
    wq1, wk1, wv1 = open_w(pc1, 1, dma=False)
    # DMA priority: the first attention group needs wq2+x2q (qT2 hc0) and
    # wk1 + x1f chunk 0 (kT1 hc0 keys 0-383); everything else trails.
    # first attention group's deps ride the otherwise-idle ACT DGE queue so
    # their completion isn't semaphore-batched behind the bulk x1f stream
    nc.scalar.dma_start(
        wq2[:], ins["wq2T"].rearrange("(a p) h -> p a h", p=128))
    nc.scalar.dma_start(
        xq8[2][:], ins["x2q8"].rearrange("(a p) l -> p a l", p=128))
    dma_w(wk1, "wk", 1)

    def mid():
        dma_w(wv1, "wv", 1)
        dma_w(wq1, "wq", 1)
        nc.sync.dma_start(
            xq8[1][:], ins["x1q8"].rearrange("(a p) l -> p a l", p=128))
        nc.sync.dma_start(
            xq[2][:], ins["x2q"].rearrange("(a p) l -> p a l", p=128))
        nc.sync.dma_start(
            xq[1][:], ins["x1q"].rearrange("(a p) l -> p a l", p=128))
    x1 = open_x(pc1, 1, mid=mid, nch=6)
    for w, wn in ((wk2, "wk"), (wv2, "wv")):
        dma_w(w, wn, 2)
    pc3 = ExitStack()
    x2 = open_x(pc3, 2, nch=6)

    # minimal pre-attention PE work; the rest of modal-1's projections are
    # fillers drained one per group during tile 0 (ordered by first use:
    # kT1-hc0 chunks gate hp0's QK sweep, va1 chunks gate its AVs, then
    # qT2-hc1..3 / kT1-hc1..3 ahead of hp1..3, then modal-2 k/v for dir 1->2)
    pq_round(pp2, 2, wq2, 0, 0)
    for lt in range(L // KT):
        k_round(pp2, 1, x1, wk1, 0, lt)
    for k in range(10):
        v_round(pp2, 1, x1, wv1, k)

    def K1(hc, lt):
        return lambda: k_round(pp2, 1, x1, wk1, hc, lt)

    def V1(k):
        return lambda: v_round(pp2, 1, x1, wv1, k)

    def Q2(hc, t):
        return lambda: pq_round(pp2, 2, wq2, hc, t)

    # ordered by first-use (1 fill/group during tile 0: qk of unit u has u
    # fills before it, av of chunk-pair cp has cp+6 given the defer-5 queue)
    pro = [V1(10), V1(11), Q2(1, 0), K1(1, 0), V1(12), V1(13), K1(1, 1),
           V1(14), K1(1, 2), V1(15), V1(16), K1(1, 3), V1(17), K1(1, 4),
           K1(1, 5), Q2(2, 0), K1(2, 0), K1(2, 1), K1(2, 2), K1(2, 3),
           K1(2, 4), K1(2, 5), Q2(3, 0), K1(3, 0), K1(3, 1), K1(3, 2),
           K1(3, 3), K1(3, 4), K1(3, 5),
           Q2(0, 1), Q2(1, 1), Q2(2, 1), Q2(3, 1)]
    fillers.extend(pro)

    # modal-1 Q projection + modal 2 k/v rounds: fillers for tiles 1-2
    fillers.append(lambda: pq_round(pp2, 1, wq1, 0, 0))
    kv2 = []
    for hc in range(HC):
        for lt in range(L // KT):
            kv2.append(lambda hc=hc, lt=lt: k_round(pp2, 2, x2, wk2, hc, lt))
    vstart = len(kv2)
    for k in range(NK):
        kv2.append(lambda k=k: v_round(pp2, 2, x2, wv2, k))
    mixed = []
    ki, vi = 0, vstart
    while ki < vstart or vi < len(kv2):
        if ki < vstart:
            mixed.append(kv2[ki]); ki += 1
            if ki < vstart:
                mixed.append(kv2[ki]); ki += 1
        if vi < len(kv2):
            mixed.append(kv2[vi]); vi += 1
    for i, t in enumerate([(0, 1), (1, 0), (1, 1), (2, 0), (2, 1), (3, 0),
                           (3, 1)]):
        mixed.insert(5 * i + 4,
                     lambda hc=t[0], t_=t[1]: pq_round(pp2, 1, wq1, hc, t_))
    fillers.extend(mixed)

    holder = {}
    fp_pool = {}
    y2t = {}
    y1t = {}

    def early_dir():
        # all modal-2 projections must be emitted before dir 1->2 reads them
        fill(len(fillers))
        pc3.close()
        pc1.close()
        pc2.close()
        holder["post"] = ctx.enter_context(
            tc.tile_pool(name="post", bufs=2, space="PSUM"))
        fp_pool["p"] = (holder["post"], "post")

    def post_fillers(m, t):
        pool = holder["post"]
        for cc in range(CC):
            fillers.append(lambda cc=cc: post_a_cc(pool, m, t, cc))
        fillers.append(lambda: post_b_stats(pool, m, t))
        for cc in range(CC):
            fillers.append(lambda cc=cc: post_b_apply(m, t, cc))

    def late_dir():
        for t in range(NT):
            post_fillers(2, t)

    def late_t1():
        pool = holder["post"]
        post_fillers(1, 0)
        for cc in range(CC):
            fillers.append(lambda cc=cc: fuse_cc(pool, 0, cc))

    def late_t2():
        pool = holder["post"]
        post_fillers(1, 1)
        for cc in range(CC):
            fillers.append(lambda cc=cc: fuse_cc(pool, 1, cc))

    # fin/bc before the post pool opens parks bc tiles in pp2's ring instead
    fp_pool["p"] = (pp2, "pp")

    tiles = [(2, 1, 0), (2, 1, 1), (2, 1, 2), (1, 2, 0), (1, 2, 1), (1, 2, 2)]
    attention_flat(tiles, {3: early_dir},
                   {3: late_dir, 4: late_t1, 5: late_t2})
    fill(len(fillers))
    pool = holder["post"]
    post_all(pool, 1, 2)
    for cc in range(CC):
        fuse_cc(pool, 2, cc)


def host_prep(inputs):
    """Precompute transposed weights / folded biases; slice per-core inputs."""
    import ml_dtypes
    bf = lambda a: np.ascontiguousarray(
        np.asarray(a, np.float32), dtype=ml_dtypes.bfloat16)
    pvals = {
        "bq1": inputs["bq1"] * WSC, "bq2": inputs["bq2"] * WSC,
        "bo1p": inputs["bo1"] + inputs["wo1"] @ inputs["bv1"],
        "bo2p": inputs["bo2"] + inputs["wo2"] @ inputs["bv2"],
        "ln1g": inputs["ln1_g"], "ln1b": inputs["ln1_b"],
        "ln2g": inputs["ln2_g"], "ln2b": inputs["ln2_b"],
    }
    bnw = inputs["bn_g"] / np.sqrt(inputs["bn_var"] + EPS)
    pvals["bnw"] = bnw
    pvals["bnb"] = (inputs["bf"] - inputs["bn_mean"]) * bnw + inputs["bn_b"]
    packed = np.zeros((128, NPARAM_COLS), np.float32)
    for nm, (off, ch) in _PARAM_SLOTS.items():
        packed[:, off:off + ch] = np.asarray(pvals[nm], np.float32).reshape(
            ch, 128).T

    f8 = lambda a: np.ascontiguousarray(
        np.asarray(a, np.float32), dtype=ml_dtypes.float8_e4m3)
    f85 = lambda a: np.ascontiguousarray(
        np.asarray(a, np.float32), dtype=ml_dtypes.float8_e5m2)
    shared = {
        "params": packed,
        "wq1T": f8(inputs["wq1"].T * WSC), "wk1T": f8(inputs["wk1"].T * WSC),
        "wv1T": f8(inputs["wv1"].T * WSC), "wq2T": f8(inputs["wq2"].T * WSC),
        "wk2T": f8(inputs["wk2"].T * WSC), "wv2T": f8(inputs["wv2"].T * WSC),
        "wo1T": f85(inputs["wo1"].T), "wo2T": f85(inputs["wo2"].T),
        "wfT": bf(inputs["wf"].T),
    }
    x1 = np.asarray(inputs["modal1_feat"], np.float32).reshape(B, C, L)
    x2 = np.asarray(inputs["modal2_feat"], np.float32).reshape(B, C, L)
    in_maps = []
    for core in range(NCORES):
        b, q = core // 4, core % 4
        m = dict(shared)
        m["x1f"] = f8(x1[b])
        m["x2f"] = f8(x2[b])
        m["x1q"] = bf(x1[b][:, LQ * q:LQ * (q + 1)])
        m["x2q"] = bf(x2[b][:, LQ * q:LQ * (q + 1)])
        m["x1q8"] = f8(x1[b][:, LQ * q:LQ * (q + 1)])
        m["x2q8"] = f8(x2[b][:, LQ * q:LQ * (q + 1)])
        in_maps.append(m)
    return in_maps


_IN_SPECS = [
    ("x1f", (C, L)), ("x2f", (C, L)), ("x1q", (C, LQ)), ("x2q", (C, LQ)),
    ("x1q8", (C, LQ)), ("x2q8", (C, LQ)),
    ("wq1T", (C, HID)), ("wk1T", (C, HID)), ("wv1T", (C, HID)),
    ("wq2T", (C, HID)), ("wk2T", (C, HID)), ("wv2T", (C, HID)),
    ("wo1T", (HID, C)), ("wo2T", (HID, C)), ("wfT", (HID, C)),
    ("params", (128, NPARAM_COLS)),
]

_BF16_INS = {"x1q", "x2q", "wfT"}
_FP8_INS = {"x1f", "x2f", "x1q8", "x2q8", "wq1T", "wk1T", "wv1T",
            "wq2T", "wk2T", "wv2T"}
_FP8E5_INS = {"wo1T", "wo2T"}


def build_program():
    nc = bacc.Bacc("TRN2", target_bir_lowering=False, debug=False)
    F8E5 = mybir.dt.float8e5
    ins = {
        name: nc.dram_tensor(
            name, list(shape),
            F8E5 if name in _FP8E5_INS else (
                FP8 if name in _FP8_INS else (
                    BF16 if name in _BF16_INS else F32)),
            kind="ExternalInput",
        ).ap()
        for name, shape in _IN_SPECS
    }
    outs = {"y": nc.dram_tensor("y", [C, LQ], F32, kind="ExternalOutput").ap()}
    with tile.TileContext(nc) as tc:
        core_kernel(tc, outs, ins)
    nc.compile()
    return nc


def _install_ntff_hook():
    """Provide antenv.axon_hooks (absent in this image) so trace=True works."""
    import sys, types
    if "antenv.axon_hooks" in sys.modules:
        return
    try:
        from trn_agent_boot.trn_boot import _ntff_profile_via_ctypes
        hook = _ntff_profile_via_ctypes("/opt/axon/libaxon_pjrt.so")
    except Exception:
        hook = None
    mod = types.ModuleType("antenv.axon_hooks")
    state = {"hook": hook}
    mod.set_axon_ntff_profile_hook = lambda h: state.__setitem__("hook", h)
    mod.get_axon_ntff_profile_hook = lambda: state["hook"]
    sys.modules["antenv.axon_hooks"] = mod


def kernel(**inputs) -> np.ndarray:
    global LAST_EXEC_NS, LAST_RESULTS
    from concourse.bass_utils import run_bass_kernel_spmd

    in_maps = host_prep(inputs)
    nc = build_program()
    trace = bool(int(os.environ.get("MMPAF_TRACE", "0")))
    if trace:
        _install_ntff_hook()
    res = run_bass_kernel_spmd(
        nc, in_maps, core_ids=list(range(NCORES)), trace=trace
    )
    LAST_EXEC_NS = res.exec_time_ns
    LAST_RESULTS = res
    out = np.empty((B, C, L), np.float32)
    for core in range(NCORES):
        b, q = core // 4, core % 4
        out[b, :, LQ * q:LQ * (q + 1)] = res.results[core]["y"]
    return out.reshape(B, C, H, W)
